# revision 8
# baseline (speedup 1.0000x reference)
"""Trainium2 Bass kernel for nn_AttentionLayer (BS=16, NA=NB=2048, NK=NV=64).

reference:
    w    = softmax_over_NA(key @ query^T / sqrt(NK))      (BS, NA, NB)
    vals = einsum('ban,bav->bnv', w, value)               (BS, NB, NV)
    out  = layernorm_over_NV(vals).transpose(0, 2, 1)     (BS, NV, NB)
    returns (out, w)

Sharding: data-parallel over BS across 8 cores (2 batches/core), no
collectives. Within a core, NB is processed in halves of 1024 columns.

Per (batch, NB-half):
  - mm1: s[a-chunk] = keyT_chunk^T @ queryT (f32r, K=64) -> PSUM
  - exp: ScalarE Exp(scale=1/8) PSUM -> SBUF e-chunk     (unnormalized)
  - mm2: acc += value_aug_chunk^T @ e-chunk (f32r, K=128) -> (65, 1024)
         row 64 of acc = column sums of e (softmax denominators)
  - tail: recip = 1/d; broadcast to 128 partitions via DMA;
          w-chunk = e * recip (DVE, in place); DMA w out.
  - norm: PE-transpose acc 128-col blocks -> (128, 65);
          bn_stats/bn_aggr over the 64 features; std via exp(0.5*ln(var))
          (keeps ScalarE on the natural_log_exp table set);
          out = (x - mean) * rsqrt-scale via fused tensor_scalar.
          The softmax denominator cancels in the layernorm (scale
          invariance), so acc is normalized without dividing by d.

Outputs per core: w (2, 2048, 2048) and vals-normalized in (NB, NV)
layout; the host transposes to (NV, NB) and concatenates.
"""

import sys

for _p in ("/opt/trn_rl_repo", "/opt/trn_rl_repo/concourse"):
    if _p not in sys.path:
        sys.path.insert(0, _p)

from contextlib import ExitStack

import numpy as np

import concourse.bass as bass
import concourse.tile as tile
from concourse import bacc, mybir
from concourse.bass_utils import run_bass_kernel_spmd
from concourse.masks import make_identity

BS, NA, NB, NK, NV = 16, 2048, 2048, 64, 64
NCORES = 8
BPC = BS // NCORES        # batches per core
NBH = NB // 2             # NB half processed at a time
NAC = NA // 128           # number of 128-row NA chunks
F32 = mybir.dt.float32
F32R = mybir.dt.float32r
AF = mybir.ActivationFunctionType

TRACE = False             # test.py flips this to profile
_cache = {}


def _build():
    nc = bacc.Bacc("TRN2", target_bir_lowering=False, debug=False,
                   num_devices=NCORES)
    kt_d = nc.dram_tensor("kt", [BPC, NK, NA], F32, kind="ExternalInput")
    qt_d = nc.dram_tensor("qt", [BPC, NK, NB], F32, kind="ExternalInput")
    va_d = nc.dram_tensor("va", [BPC, NAC, 128, NV + 1], F32,
                          kind="ExternalInput")
    w_d = nc.dram_tensor("w_out", [BPC, NA, NB], F32, kind="ExternalOutput")
    o_d = nc.dram_tensor("o_out", [BPC, NB, NV], F32, kind="ExternalOutput")
    rb_d = nc.dram_tensor("rb_scratch", [BPC * 2, NBH], F32)

    with ExitStack() as ctx:
        tc = ctx.enter_context(tile.TileContext(nc))
        consts = ctx.enter_context(tc.tile_pool(name="consts", bufs=1))
        inp = ctx.enter_context(tc.tile_pool(name="inp", bufs=2))
        epool = ctx.enter_context(tc.tile_pool(name="e", bufs=18))
        wpool = ctx.enter_context(tc.tile_pool(name="w", bufs=6))
        tailp = ctx.enter_context(tc.tile_pool(name="tail", bufs=2))
        small = ctx.enter_context(tc.tile_pool(name="small", bufs=4))
        ps_s = ctx.enter_context(tc.tile_pool(name="ps_s", bufs=2, space="PSUM"))
        ps_acc = ctx.enter_context(tc.tile_pool(name="ps_acc", bufs=1, space="PSUM"))
        ps_tp = ctx.enter_context(tc.tile_pool(name="ps_tp", bufs=2, space="PSUM"))

        ident = consts.tile([128, 128], F32)
        make_identity(nc, ident)

        for b in range(BPC):
            ktr = inp.tile([NK, NA], F32R, tag="kt")
            nc.gpsimd.dma_start(out=ktr, in_=kt_d[b])
            qtr = inp.tile([NK, NB], F32R, tag="qt")
            nc.gpsimd.dma_start(out=qtr, in_=qt_d[b])
            var = inp.tile([128, NAC, NV + 1], F32R, tag="va")
            nc.gpsimd.dma_start(out=var, in_=va_d[b].rearrange("c p v -> p c v"))

            for h in range(2):
                hs = h * NBH
                acc = ps_acc.tile([NV + 1, NBH], F32, tag="acc")
                es = []
                for a in range(NAC):
                    s = ps_s.tile([128, NBH], F32, tag="s")
                    for q in range(2):
                        nc.tensor.matmul(
                            s[:, q * 512:(q + 1) * 512],
                            lhsT=ktr[:, a * 128:(a + 1) * 128],
                            rhs=qtr[:, hs + q * 512: hs + (q + 1) * 512],
                            start=True, stop=True)
                    e = epool.tile([128, NBH], F32R, tag="e")
                    nc.scalar.activation(e, s, AF.Exp, scale=float(1.0 / np.sqrt(NK)))
                    for q in range(2):
                        nc.tensor.matmul(
                            acc[:, q * 512:(q + 1) * 512],
                            lhsT=var[:, a, :],
                            rhs=e[:, q * 512:(q + 1) * 512],
                            start=(a == 0), stop=(a == NAC - 1))
                    es.append(e)

                # ---- softmax tail: divide e by column sums, store w ----
                va_s = tailp.tile([NV + 1, NBH], F32, tag="vas")
                nc.scalar.copy(va_s, acc)
                recip = small.tile([1, NBH], F32, tag="recip")
                nc.vector.reciprocal(recip, va_s[NV:NV + 1, :])
                rb = tailp.tile([128, NBH], F32, tag="rb")
                nc.sync.dma_start(out=rb_d[b * 2 + h:b * 2 + h + 1, :], in_=recip[0:1, :])
                nc.sync.dma_start(
                    out=rb, in_=rb_d[b * 2 + h:b * 2 + h + 1, :].partition_broadcast(128))
                for a in range(NAC):
                    wt = wpool.tile([128, NBH], F32, tag="w")
                    nc.vector.tensor_mul(wt, es[a].bitcast(F32), rb)
                    nc.sync.dma_start(
                        out=w_d[b, a * 128:(a + 1) * 128, hs:hs + NBH],
                        in_=wt)

                # ---- layernorm of vals (scale-invariant in d) ----
                bigts = tailp.tile([128, 8, NV + 1], F32, tag="bigts")
                mvs = small.tile([128, 8, 2], F32, tag="mvs")
                for j in range(8):
                    tp = ps_tp.tile([128, NV + 1], F32, tag="tp")
                    nc.tensor.transpose(tp, va_s[:, j * 128:(j + 1) * 128],
                                        ident[0:NV + 1, 0:NV + 1])
                    nc.vector.tensor_copy(bigts[:, j, :], tp)
                    st = small.tile([128, 6], F32, tag="st")
                    nc.vector.bn_stats(st, bigts[:, j, 0:NV])
                    nc.vector.bn_aggr(mvs[:, j, :], st)
                # unbiased std: exp(0.5 * ln(var * NV/(NV-1))) == 1/rsqrt-scale
                lt = small.tile([128, 8], F32, tag="lt")
                nc.scalar.activation(lt, mvs[:, :, 1], AF.Ln,
                                     scale=float(NV / (NV - 1)))
                sc = small.tile([128, 8], F32, tag="sc")
                nc.scalar.activation(sc, lt, AF.Exp, scale=-0.5)
                on = tailp.tile([128, 8, NV], F32, tag="on")
                for j in range(8):
                    nc.vector.tensor_scalar(
                        on[:, j, :], bigts[:, j, 0:NV],
                        scalar1=mvs[:, j, 0:1], scalar2=sc[:, j:j + 1],
                        op0=mybir.AluOpType.subtract,
                        op1=mybir.AluOpType.mult)
                nc.sync.dma_start(
                    out=o_d[b, hs:hs + NBH, :].rearrange("(j p) v -> p j v", p=128),
                    in_=on)

    nc.compile()
    return nc


def _prep_core_inputs(key, query, value, core):
    """Host-side shard + layout prep for one core (2 batches)."""
    b0 = core * BPC
    k = key[b0:b0 + BPC, :, 0, :]          # (BPC, NA, NK)
    q = query[b0:b0 + BPC, 0, :, :]        # (BPC, NB, NK)
    v = value[b0:b0 + BPC]                 # (BPC, NA, NV)
    kt = np.ascontiguousarray(k.transpose(0, 2, 1))   # (BPC, NK, NA)
    qt = np.ascontiguousarray(q.transpose(0, 2, 1))   # (BPC, NK, NB)
    va = np.concatenate(
        [v, np.ones((BPC, NA, 1), np.float32)], axis=2)  # (BPC, NA, NV+1)
    va = np.ascontiguousarray(va.reshape(BPC, NAC, 128, NV + 1))
    return {"kt": kt, "qt": qt, "va": va}


def kernel(key, query, value):
    key = np.asarray(key, dtype=np.float32)
    query = np.asarray(query, dtype=np.float32)
    value = np.asarray(value, dtype=np.float32)

    if "nc" not in _cache:
        _cache["nc"] = _build()
    nc = _cache["nc"]

    in_maps = [_prep_core_inputs(key, query, value, c) for c in range(NCORES)]
    res = run_bass_kernel_spmd(nc, in_maps, list(range(NCORES)), trace=TRACE)
    _cache["last_results"] = res

    w = np.concatenate([res.results[c]["w_out"] for c in range(NCORES)], axis=0)
    o_nbnv = np.concatenate([res.results[c]["o_out"] for c in range(NCORES)],
                            axis=0)                    # (BS, NB, NV)
    out = np.ascontiguousarray(o_nbnv.transpose(0, 2, 1))  # (BS, NV, NB)
    return out, w
